# revision 1
# baseline (speedup 1.0000x reference)
"""Trainium2 Bass kernel for windowed (sparse) cross-attention.

Computation (per batch row b of x:(B=2048, N=64, D=512)):
  q/k/v = x @ Wq/Wk/Wv, split into 8 heads of dim 64.
  13 temporal windows of size 16, stride 4 over N=64; softmax attention within
  each window; overlapping window outputs are accumulated and divided by the
  per-position window count; out = value @ Wout + bout.

Strategy (pure data parallel over 8 NeuronCores, batch-sharded):
  - Host pre-transposes the x shard to xT (D, T) and casts operands to bf16.
  - Projections produce qT/kT (inner-on-partitions) and v (tokens-on-partitions).
  - Scores S'[m,n] = k_m . q_n are computed per (2-batch x 2-head) "quad" with
    K=64 matmuls using tile_position row halves; the full 64x64 score block per
    pair is materialized (windows are diagonal 16x16 sub-blocks of it).
  - Window softmax is linear-algebra-ified: with E = exp(S*scale),
      window sums   s[w, n] = (MaskStack^T @ E)        (one matmul)
      R'[m, n] = MaskStack @ (U * 1/s)                 (one matmul)
      P'[m, n] = E * R'                                (elementwise)
    where U[w,n] = 1[n in window w] / cnt[n].  Then value^T = v^T-contracted
    AV matmuls over P' columns.  This makes the entire softmax+window-overlap
    accumulation 2 small matmuls + 3 vector ops per 2-quad unit.
  - Output projection consumes value^T chunks as stationary operands and
    produces the output in natural (token, D) layout; bias added on DVE.
"""

import sys

if "/opt/trn_rl_repo" not in sys.path:
    sys.path.insert(0, "/opt/trn_rl_repo")

import numpy as np
import ml_dtypes

import concourse.bass as bass
import concourse.tile as tile
from concourse import mybir
from concourse.bass_utils import run_bass_kernel_spmd

BF16 = mybir.dt.bfloat16
F32 = mybir.dt.float32
NP_BF16 = ml_dtypes.bfloat16

# Problem constants (hardcoded per contract)
B, N, D = 2048, 64, 512
NCORES = 8
BC = B // NCORES          # batch rows per core
T_FULL = BC * N           # tokens per core = 16384
HEADS, DH = 8, 64
WINDOW, STRIDE, NW = 16, 4, 13
SCALE = DH ** -0.5
TB = 512                  # tokens per block (8 batch rows)

# stash for test harness introspection
last_results = None


def _split_waits(nc, keep=1):
    """walrus in this toolchain supports only one embedded sync wait per
    instruction; hoist excess waits onto standalone EventSemaphore
    instructions on the same engine queue (FIFO => executes first)."""
    ctr = 0
    for f in nc.m.functions:
        for blk in f.blocks:
            il = blk.instructions
            out = []
            changed = False
            for inst in il:
                si = inst.sync_info
                if si is not None and len(si.on_wait) > keep:
                    waits = list(si.on_wait)
                    SyncInfo = type(si)
                    for w in waits[:-keep]:
                        evs = mybir.InstEventSemaphore(
                            name=f"WSPLIT-{ctr}", ins=[], outs=[]
                        )
                        ctr += 1
                        evs.engine = inst.engine
                        evs.sync_info = SyncInfo(on_wait=[w], on_update=[])
                        out.append(evs)
                    inst.sync_info = SyncInfo(
                        on_wait=waits[-keep:], on_update=list(si.on_update)
                    )
                    changed = True
                out.append(inst)
            if changed:
                il[:] = out
    return ctr


def _window_consts():
    idx = np.arange(NW)[:, None] * STRIDE + np.arange(WINDOW)[None, :]
    cnt = np.zeros(N, dtype=np.float64)
    np.add.at(cnt, idx, 1.0)
    member = np.zeros((N, NW), dtype=np.float64)  # member[m, w] = m in window w
    for w in range(NW):
        member[idx[w], w] = 1.0
    mask_s = np.zeros((128, 26), dtype=np.float64)
    mask_s[:64, :13] = member
    mask_s[64:, 13:] = member
    mask_t = mask_s.T.copy()
    u = np.zeros((26, 512), dtype=np.float64)
    for j in range(512):
        s = ((j % 256) // 64) % 2
        n = j % 64
        u[s * 13:(s + 1) * 13, j] = member[n] / cnt[n]
    return (
        mask_s.astype(NP_BF16),
        mask_t.astype(NP_BF16),
        u.astype(np.float32),
    )


def build_program(T=T_FULL):
    nc = bass.Bass()
    xt_d = nc.dram_tensor("xt", [D, T], BF16, kind="ExternalInput")
    wq_d = nc.dram_tensor("wq", [128, 4, D], BF16, kind="ExternalInput")
    wk_d = nc.dram_tensor("wk", [128, 4, D], BF16, kind="ExternalInput")
    wv_d = nc.dram_tensor("wv", [128, 4, D], BF16, kind="ExternalInput")
    wo_d = nc.dram_tensor("wo", [128, 4, D], BF16, kind="ExternalInput")
    bo_d = nc.dram_tensor("bo", [128, D], F32, kind="ExternalInput")
    ms_d = nc.dram_tensor("ms", [128, 26], BF16, kind="ExternalInput")
    mt_d = nc.dram_tensor("mt", [26, 128], BF16, kind="ExternalInput")
    u_d = nc.dram_tensor("u", [26, 512], F32, kind="ExternalInput")
    out_d = nc.dram_tensor("out", [T, D], F32, kind="ExternalOutput")

    NB = T // TB
    EXP = mybir.ActivationFunctionType.Exp

    with tile.TileContext(nc) as tc:
        with (
            tc.tile_pool(name="consts", bufs=1) as consts,
            tc.tile_pool(name="xtp", bufs=8) as xt_pool,
            tc.tile_pool(name="qkp", bufs=16) as qk_pool,
            tc.tile_pool(name="vp", bufs=8) as v_pool,
            tc.tile_pool(name="ep", bufs=4) as e_pool,
            tc.tile_pool(name="rcp", bufs=4) as rc_pool,
            tc.tile_pool(name="pp", bufs=4) as p_pool,
            tc.tile_pool(name="vtp", bufs=8) as vt_pool,
            tc.tile_pool(name="op", bufs=4) as out_pool,
            tc.tile_pool(name="ps_proj", bufs=2, space="PSUM") as ps_proj,
            tc.tile_pool(name="ps_s", bufs=2, space="PSUM") as ps_s,
            tc.tile_pool(name="ps_w", bufs=1, space="PSUM") as ps_w,
            tc.tile_pool(name="ps_r", bufs=1, space="PSUM") as ps_r,
            tc.tile_pool(name="ps_av", bufs=2, space="PSUM") as ps_av,
        ):
            wq_t = consts.tile([128, 4, D], BF16, tag="wq")
            nc.sync.dma_start(wq_t[:], wq_d[:])
            wk_t = consts.tile([128, 4, D], BF16, tag="wk")
            nc.sync.dma_start(wk_t[:], wk_d[:])
            wv_t = consts.tile([128, 4, D], BF16, tag="wv")
            nc.sync.dma_start(wv_t[:], wv_d[:])
            wo_t = consts.tile([128, 4, D], BF16, tag="wo")
            nc.sync.dma_start(wo_t[:], wo_d[:])
            bo_t = consts.tile([128, D], F32, tag="bo")
            nc.sync.dma_start(bo_t[:], bo_d[:])
            ms_t = consts.tile([128, 26], BF16, tag="ms")
            nc.sync.dma_start(ms_t[:], ms_d[:])
            mt_t = consts.tile([26, 128], BF16, tag="mt")
            nc.sync.dma_start(mt_t[:], mt_d[:])
            u_t = consts.tile([26, 512], F32, tag="u")
            nc.sync.dma_start(u_t[:], u_d[:])

            for blk in range(NB):
                t0 = blk * TB

                # ---- load xT tiles (D on partitions, 4 chunks) ----
                xts = []
                for kc in range(4):
                    xt_t = xt_pool.tile([128, TB], BF16, tag="xt")
                    nc.sync.dma_start(
                        xt_t[:], xt_d[kc * 128:(kc + 1) * 128, t0:t0 + TB]
                    )
                    xts.append(xt_t)

                # ---- qT / kT projections, stored as per-head-half tiles
                # [64, TB] at base partition 0 (avoids partition-offset
                # matmul operands, which wedge this hardware) ----
                qts, kts = [], []
                for wt, lst in ((wq_t, qts), (wk_t, kts)):
                    for c in range(4):
                        ps = ps_proj.tile([128, TB], F32, tag="pp")
                        for kc in range(4):
                            nc.tensor.matmul(
                                ps[:],
                                wt[:, kc, c * 128:(c + 1) * 128],
                                xts[kc][:],
                                start=(kc == 0),
                                stop=(kc == 3),
                            )
                        halves = []
                        for hh in range(2):
                            sb = qk_pool.tile([64, TB], BF16, tag="qk")
                            nc.scalar.copy(sb[:], ps[hh * 64:(hh + 1) * 64, :])
                            halves.append(sb)
                        lst.append(halves)

                # ---- v projection: natural layout [128 tokens, 512 i] ----
                vts = []
                for tt in range(4):
                    ps = ps_proj.tile([128, 512], F32, tag="pp")
                    for kc in range(4):
                        nc.tensor.matmul(
                            ps[:],
                            xts[kc][:, tt * 128:(tt + 1) * 128],
                            wv_t[:, kc, :],
                            start=(kc == 0),
                            stop=(kc == 3),
                        )
                    sb = v_pool.tile([128, 512], BF16, tag="vv")
                    nc.vector.tensor_copy(sb[:], ps[:])
                    vts.append(sb)

                # ---- attention per chunk (2 heads) ----
                vt_out = []
                for c in range(4):
                    qc, kc_t = qts[c], kts[c]
                    av = ps_av.tile([128, 512], F32, tag="av")
                    for tb2 in range(2):
                        # unit: 2 quads (each quad = 2 batch rows x 2 heads)
                        sp = ps_s.tile([128, 512], F32, tag="sp")
                        for qd in range(2):
                            tb = tb2 * 2 + qd
                            for hh in range(2):
                                tcols = slice(tb * 128, (tb + 1) * 128)
                                o = sp[:, qd * 256 + hh * 128:
                                       qd * 256 + (hh + 1) * 128]
                                nc.tensor.matmul(
                                    o, kc_t[hh][:, tcols], qc[hh][:, tcols],
                                    start=True, stop=True,
                                )
                        eu = e_pool.tile([128, 512], BF16, tag="eu")
                        nc.scalar.activation(eu[:], sp[:], EXP, scale=float(SCALE))
                        # window sums for all 4 pairs: [26, 512]
                        sw = ps_w.tile([128, 512], F32, tag="sw")
                        nc.tensor.matmul(sw[:26, :], ms_t[:], eu[:], start=True, stop=True)
                        rc = rc_pool.tile([26, 512], F32, tag="rc")
                        nc.vector.reciprocal(rc[:], sw[:26, :])
                        rcu = rc_pool.tile([26, 512], BF16, tag="rcu")
                        nc.vector.tensor_mul(rcu[:], rc[:], u_t[:])
                        rp = ps_r.tile([128, 512], F32, tag="rp")
                        nc.tensor.matmul(rp[:], mt_t[:], rcu[:], start=True, stop=True)
                        pu = p_pool.tile([128, 512], BF16, tag="pu")
                        nc.vector.tensor_mul(pu[:], eu[:], rp[:])
                        # AV: value^T quad blocks -> av[:, tb*128 + ...]
                        for qd in range(2):
                            tb = tb2 * 2 + qd
                            for hh in range(2):
                                lhsT = vts[tb][
                                    :, c * 128 + hh * 64: c * 128 + hh * 64 + 64
                                ]
                                rhs = pu[:, qd * 256 + hh * 128:
                                         qd * 256 + (hh + 1) * 128]
                                o = av[hh * 64:(hh + 1) * 64,
                                       tb * 128:(tb + 1) * 128]
                                nc.tensor.matmul(o, lhsT, rhs, start=True, stop=True)
                    vt = vt_pool.tile([128, 512], BF16, tag="vt")
                    nc.scalar.copy(vt[:], av[:])
                    vt_out.append(vt)

                # ---- output projection + bias ----
                for tt in range(4):
                    ps = ps_proj.tile([128, 512], F32, tag="pp")
                    for c in range(4):
                        nc.tensor.matmul(
                            ps[:],
                            vt_out[c][:, tt * 128:(tt + 1) * 128],
                            wo_t[:, c, :],
                            start=(c == 0),
                            stop=(c == 3),
                        )
                    ob = out_pool.tile([128, 512], F32, tag="ob")
                    nc.vector.tensor_add(ob[:], ps[:], bo_t[:])
                    nc.sync.dma_start(
                        out_d[t0 + tt * 128: t0 + (tt + 1) * 128, :], ob[:]
                    )
    return nc


def _prep_shared(Wq, Wk, Wv, Wout, bout):
    def warr(w):
        return np.ascontiguousarray(
            w.astype(np.float32).reshape(4, 128, D).transpose(1, 0, 2)
        ).astype(NP_BF16)

    mask_s, mask_t, u = _window_consts()
    return {
        "wq": warr(Wq),
        "wk": warr(Wk),
        "wv": warr(Wv),
        "wo": warr(Wout),
        "bo": np.ascontiguousarray(
            np.broadcast_to(bout.astype(np.float32), (128, D))
        ),
        "ms": mask_s,
        "mt": mask_t,
        "u": u,
    }


def kernel(x, Wq, Wk, Wv, Wout, bout):
    global last_results
    x = np.asarray(x, dtype=np.float32)
    shared = _prep_shared(
        np.asarray(Wq), np.asarray(Wk), np.asarray(Wv),
        np.asarray(Wout), np.asarray(bout),
    )
    in_maps = []
    for ci in range(NCORES):
        xs = x[ci * BC:(ci + 1) * BC].reshape(T_FULL, D)
        xt = np.ascontiguousarray(xs.T).astype(NP_BF16)
        in_maps.append({"xt": xt, **shared})

    nc = build_program(T_FULL)
    _split_waits(nc)
    res = run_bass_kernel_spmd(nc, in_maps, list(range(NCORES)))
    last_results = res
    outs = [
        res.results[ci]["out"].astype(np.float32).reshape(BC, N, D)
        for ci in range(NCORES)
    ]
    return np.concatenate(outs, axis=0)



# revision 6
# speedup vs baseline: 1.4076x; 1.4076x over previous
"""Trainium2 Bass kernel for windowed (sparse) cross-attention.

Computation (per batch row b of x:(B=2048, N=64, D=512)):
  q/k/v = x @ Wq/Wk/Wv, split into 8 heads of dim 64.
  13 temporal windows of size 16, stride 4 over N=64; softmax attention within
  each window; overlapping window outputs are accumulated and divided by the
  per-position window count; out = value @ Wout + bout.

Strategy (pure data parallel over 8 NeuronCores, batch-sharded):
  - Host pre-transposes the x shard to xT (D, T) and casts operands to bf16.
  - Projections produce qT/kT (inner-on-partitions) and v (tokens-on-partitions).
  - Scores S'[m,n] = k_m . q_n are computed per (2-batch x 2-head) "quad" with
    K=64 matmuls; the full 64x64 score block per pair is materialized (windows
    are diagonal 16x16 sub-blocks of it).
  - Window softmax is linear-algebra-ified: with E = exp(S*scale),
      window sums   s[w, n] = (MaskStack^T @ E)        (one matmul per unit,
                                                        packed 4 units/bank)
      1/s           via exp(-ln(s)) on the ACT engine  (ln+exp share one act
                                                        table set: no swaps)
      R'[m, n] = MaskStack @ (U * 1/s)                 (one matmul)
      P'[m, n] = E * R'                                (elementwise)
    where U[w,n] = 1[n in window w] / cnt[n].  Then value^T = v^T-contracted
    AV matmuls over P' columns.
  - Four units' window sums are packed into one PSUM bank at 32-aligned
    partition offsets (mask padded to 32 dummy columns so every partition
    holds a valid positive sum); ln/exp then run once per 4 units.  The
    U-multiply writes per-unit tiles at base partition 0 so matmul operands
    never sit at a partition offset (offset operands wedge this hardware).
  - Output projection consumes value^T chunks as stationary operands and
    produces the output in natural (token, D) layout; bias added on DVE.
"""

import sys

if "/opt/trn_rl_repo" not in sys.path:
    sys.path.insert(0, "/opt/trn_rl_repo")

import numpy as np
import ml_dtypes

import concourse.bass as bass
import concourse.tile as tile
from concourse import mybir
from concourse.bass_utils import run_bass_kernel_spmd

BF16 = mybir.dt.bfloat16
F32 = mybir.dt.float32
NP_BF16 = ml_dtypes.bfloat16

# Problem constants (hardcoded per contract)
B, N, D = 2048, 64, 512
NCORES = 8
BC = B // NCORES          # batch rows per core
T_FULL = BC * N           # tokens per core = 16384
HEADS, DH = 8, 64
WINDOW, STRIDE, NW = 16, 4, 13
SCALE = DH ** -0.5
TB = 512                  # tokens per block (8 batch rows)

# stash for test harness introspection
last_results = None


def _split_waits(nc, keep=1):
    """walrus in this toolchain supports only one embedded sync wait per
    instruction; hoist excess waits onto standalone EventSemaphore
    instructions on the same engine queue (FIFO => executes first)."""
    ctr = 0
    for f in nc.m.functions:
        for blk in f.blocks:
            il = blk.instructions
            out = []
            changed = False
            for inst in il:
                si = inst.sync_info
                if si is not None and len(si.on_wait) > keep:
                    waits = list(si.on_wait)
                    SyncInfo = type(si)
                    for w in waits[:-keep]:
                        evs = mybir.InstEventSemaphore(
                            name=f"WSPLIT-{ctr}", ins=[], outs=[]
                        )
                        ctr += 1
                        evs.engine = inst.engine
                        evs.sync_info = SyncInfo(on_wait=[w], on_update=[])
                        out.append(evs)
                    inst.sync_info = SyncInfo(
                        on_wait=waits[-keep:], on_update=list(si.on_update)
                    )
                    changed = True
                out.append(inst)
            if changed:
                il[:] = out
    return ctr


def _window_consts():
    idx = np.arange(NW)[:, None] * STRIDE + np.arange(WINDOW)[None, :]
    cnt = np.zeros(N, dtype=np.float64)
    np.add.at(cnt, idx, 1.0)
    member = np.zeros((N, NW), dtype=np.float64)  # member[m, w] = m in window w
    for w in range(NW):
        member[idx[w], w] = 1.0
    # 32-column mask: 26 real windows + 6 dummy copies of window 0 so that a
    # packed 4-unit [128, 512] window-sum bank has every partition written
    # with a valid positive sum (safe for ln).
    mask32 = np.zeros((128, 32), dtype=np.float64)
    mask32[:64, :13] = member
    mask32[64:, 13:26] = member
    for j in range(26, 32):
        mask32[:, j] = mask32[:, 0]
    mask_t = np.zeros((26, 128), dtype=np.float64)
    mask_t[:13, :64] = member.T
    mask_t[13:, 64:] = member.T
    u = np.zeros((26, 512), dtype=np.float64)
    for j in range(512):
        s = ((j % 256) // 64) % 2
        n = j % 64
        u[s * 13:(s + 1) * 13, j] = member[n] / cnt[n]
    # replicate u into each 32-aligned partition band: DVE tensor_tensor
    # requires both SBUF inputs at the same base partition
    u4 = np.zeros((128, 512), dtype=np.float64)
    for i in range(4):
        u4[i * 32:i * 32 + 26] = u
    return (
        mask32.astype(NP_BF16),
        mask_t.astype(NP_BF16),
        u4.astype(NP_BF16),
    )


def build_program(T=T_FULL):
    nc = bass.Bass()
    xt_d = nc.dram_tensor("xt", [D, T], BF16, kind="ExternalInput")
    wq_d = nc.dram_tensor("wq", [128, 4, D], BF16, kind="ExternalInput")
    wk_d = nc.dram_tensor("wk", [128, 4, D], BF16, kind="ExternalInput")
    wv_d = nc.dram_tensor("wv", [128, 4, D], BF16, kind="ExternalInput")
    wo_d = nc.dram_tensor("wo", [128, 4, D], BF16, kind="ExternalInput")
    bo_d = nc.dram_tensor("bo", [128, D], F32, kind="ExternalInput")
    ms_d = nc.dram_tensor("ms", [128, 32], BF16, kind="ExternalInput")
    mt_d = nc.dram_tensor("mt", [26, 128], BF16, kind="ExternalInput")
    u_d = nc.dram_tensor("u", [128, 512], BF16, kind="ExternalInput")
    out_d = nc.dram_tensor("out", [T, D], F32, kind="ExternalOutput")

    NB = T // TB
    EXP = mybir.ActivationFunctionType.Exp
    LN = mybir.ActivationFunctionType.Ln

    with tile.TileContext(nc) as tc:
        with (
            tc.tile_pool(name="consts", bufs=1) as consts,
            tc.tile_pool(name="xtp", bufs=8) as xt_pool,
            tc.tile_pool(name="qkp", bufs=16) as qk_pool,
            tc.tile_pool(name="vp", bufs=8) as v_pool,
            tc.tile_pool(name="ep", bufs=6) as e_pool,
            tc.tile_pool(name="lnp", bufs=2) as ln_pool,
            tc.tile_pool(name="rcp", bufs=8) as rc_pool,
            tc.tile_pool(name="pp", bufs=4) as p_pool,
            tc.tile_pool(name="vtp", bufs=8) as vt_pool,
            tc.tile_pool(name="op", bufs=4) as out_pool,
            tc.tile_pool(name="ps_proj", bufs=2, space="PSUM") as ps_proj,
            tc.tile_pool(name="ps_s", bufs=2, space="PSUM") as ps_s,
            tc.tile_pool(name="ps_w", bufs=1, space="PSUM") as ps_w,
            tc.tile_pool(name="ps_r", bufs=1, space="PSUM") as ps_r,
            tc.tile_pool(name="ps_av", bufs=2, space="PSUM") as ps_av,
        ):
            wq_t = consts.tile([128, 4, D], BF16, tag="wq")
            nc.sync.dma_start(wq_t[:], wq_d[:])
            wk_t = consts.tile([128, 4, D], BF16, tag="wk")
            nc.sync.dma_start(wk_t[:], wk_d[:])
            wv_t = consts.tile([128, 4, D], BF16, tag="wv")
            nc.sync.dma_start(wv_t[:], wv_d[:])
            wo_t = consts.tile([128, 4, D], BF16, tag="wo")
            nc.sync.dma_start(wo_t[:], wo_d[:])
            bo_t = consts.tile([128, D], F32, tag="bo")
            nc.sync.dma_start(bo_t[:], bo_d[:])
            ms_t = consts.tile([128, 32], BF16, tag="ms")
            nc.sync.dma_start(ms_t[:], ms_d[:])
            mt_t = consts.tile([26, 128], BF16, tag="mt")
            nc.sync.dma_start(mt_t[:], mt_d[:])
            u_t = consts.tile([128, 512], BF16, tag="u")
            nc.sync.dma_start(u_t[:], u_d[:])

            for blk in range(NB):
                t0 = blk * TB

                # ---- load xT tiles (D on partitions, 4 chunks) ----
                xts = []
                for kc in range(4):
                    xt_t = xt_pool.tile([128, TB], BF16, tag="xt")
                    nc.sync.dma_start(
                        xt_t[:], xt_d[kc * 128:(kc + 1) * 128, t0:t0 + TB]
                    )
                    xts.append(xt_t)

                # ---- qT / kT projections, stored as per-head-half tiles
                # [64, TB] at base partition 0 (avoids partition-offset
                # matmul operands, which wedge this hardware) ----
                qts, kts = [], []
                for wt, lst in ((wq_t, qts), (wk_t, kts)):
                    for c in range(4):
                        ps = ps_proj.tile([128, TB], F32, tag="pp")
                        for kc in range(4):
                            nc.tensor.matmul(
                                ps[:],
                                wt[:, kc, c * 128:(c + 1) * 128],
                                xts[kc][:],
                                start=(kc == 0),
                                stop=(kc == 3),
                            )
                        halves = []
                        for hh in range(2):
                            sb = qk_pool.tile([64, TB], BF16, tag="qk")
                            nc.scalar.copy(sb[:], ps[hh * 64:(hh + 1) * 64, :])
                            halves.append(sb)
                        lst.append(halves)

                # ---- v projection: natural layout [128 tokens, 512 i] ----
                vts = []
                for tt in range(4):
                    ps = ps_proj.tile([128, 512], F32, tag="pp")
                    for kc in range(4):
                        nc.tensor.matmul(
                            ps[:],
                            xts[kc][:, tt * 128:(tt + 1) * 128],
                            wv_t[:, kc, :],
                            start=(kc == 0),
                            stop=(kc == 3),
                        )
                    sb = v_pool.tile([128, 512], BF16, tag="vv")
                    nc.vector.tensor_copy(sb[:], ps[:])
                    vts.append(sb)

                # ---- attention, two groups of 4 units (unit = 2 token-pairs
                # x 2 heads); each group spans 2 head-chunks ----
                vt_out = [None] * 4
                for g in range(2):
                    units = [(2 * g + (ui >> 1), ui & 1) for ui in range(4)]
                    swb = ps_w.tile([128, 512], F32, tag="sw")
                    eus = []
                    for ui, (c, tb2) in enumerate(units):
                        qc, kc_t = qts[c], kts[c]
                        sp = ps_s.tile([128, 512], F32, tag="sp")
                        for qd in range(2):
                            tb = tb2 * 2 + qd
                            for hh in range(2):
                                tcols = slice(tb * 128, (tb + 1) * 128)
                                o = sp[:, qd * 256 + hh * 128:
                                       qd * 256 + (hh + 1) * 128]
                                nc.tensor.matmul(
                                    o, kc_t[hh][:, tcols], qc[hh][:, tcols],
                                    start=True, stop=True,
                                )
                        eu = e_pool.tile([128, 512], BF16, tag="eu")
                        nc.scalar.activation(eu[:], sp[:], EXP, scale=float(SCALE))
                        eus.append(eu)
                        # window sums for this unit -> 32-aligned partition
                        # band of the shared bank
                        nc.tensor.matmul(
                            swb[ui * 32:(ui + 1) * 32, :], ms_t[:], eu[:],
                            start=True, stop=True,
                            tile_position=(0, ui * 32),
                        )
                    # 1/s for all 4 units: exp(-ln(s)); both funcs live in the
                    # same ACT table set, so no table reloads
                    lt = ln_pool.tile([128, 512], F32, tag="lt")
                    nc.scalar.activation(lt[:], swb[:], LN)
                    et = ln_pool.tile([128, 512], BF16, tag="et")
                    nc.scalar.activation(et[:], lt[:], EXP, scale=-1.0)

                    av_cur = None
                    for ui, (c, tb2) in enumerate(units):
                        rcu = rc_pool.tile([26, 512], BF16, tag="rcu")
                        nc.vector.tensor_mul(
                            rcu[:], et[ui * 32:ui * 32 + 26, :],
                            u_t[ui * 32:ui * 32 + 26, :],
                        )
                        rp = ps_r.tile([128, 512], F32, tag="rp")
                        nc.tensor.matmul(rp[:], mt_t[:], rcu[:], start=True, stop=True)
                        pu = p_pool.tile([128, 512], BF16, tag="pu")
                        nc.vector.tensor_mul(pu[:], eus[ui][:], rp[:])
                        if tb2 == 0:
                            av_cur = ps_av.tile([128, 512], F32, tag="av")
                        av = av_cur
                        for qd in range(2):
                            tb = tb2 * 2 + qd
                            for hh in range(2):
                                lhsT = vts[tb][
                                    :, c * 128 + hh * 64: c * 128 + hh * 64 + 64
                                ]
                                rhs = pu[:, qd * 256 + hh * 128:
                                         qd * 256 + (hh + 1) * 128]
                                o = av[hh * 64:(hh + 1) * 64,
                                       tb * 128:(tb + 1) * 128]
                                nc.tensor.matmul(o, lhsT, rhs, start=True, stop=True)
                        if tb2 == 1:
                            vt = vt_pool.tile([128, 512], BF16, tag="vt")
                            nc.scalar.copy(vt[:], av[:])
                            vt_out[c] = vt

                # ---- output projection + bias ----
                for tt in range(4):
                    ps = ps_proj.tile([128, 512], F32, tag="pp")
                    for c in range(4):
                        nc.tensor.matmul(
                            ps[:],
                            vt_out[c][:, tt * 128:(tt + 1) * 128],
                            wo_t[:, c, :],
                            start=(c == 0),
                            stop=(c == 3),
                        )
                    ob = out_pool.tile([128, 512], F32, tag="ob")
                    nc.vector.tensor_add(ob[:], ps[:], bo_t[:])
                    nc.sync.dma_start(
                        out_d[t0 + tt * 128: t0 + (tt + 1) * 128, :], ob[:]
                    )
    return nc


def _prep_shared(Wq, Wk, Wv, Wout, bout):
    def warr(w):
        return np.ascontiguousarray(
            w.astype(np.float32).reshape(4, 128, D).transpose(1, 0, 2)
        ).astype(NP_BF16)

    mask32, mask_t, u = _window_consts()
    return {
        "wq": warr(Wq),
        "wk": warr(Wk),
        "wv": warr(Wv),
        "wo": warr(Wout),
        "bo": np.ascontiguousarray(
            np.broadcast_to(bout.astype(np.float32), (128, D))
        ),
        "ms": mask32,
        "mt": mask_t,
        "u": u,
    }


def kernel(x, Wq, Wk, Wv, Wout, bout):
    global last_results
    x = np.asarray(x, dtype=np.float32)
    shared = _prep_shared(
        np.asarray(Wq), np.asarray(Wk), np.asarray(Wv),
        np.asarray(Wout), np.asarray(bout),
    )
    in_maps = []
    for ci in range(NCORES):
        xs = x[ci * BC:(ci + 1) * BC].reshape(T_FULL, D)
        xt = np.ascontiguousarray(xs.T).astype(NP_BF16)
        in_maps.append({"xt": xt, **shared})

    nc = build_program(T_FULL)
    _split_waits(nc)
    res = run_bass_kernel_spmd(nc, in_maps, list(range(NCORES)))
    last_results = res
    outs = [
        res.results[ci]["out"].astype(np.float32).reshape(BC, N, D)
        for ci in range(NCORES)
    ]
    return np.concatenate(outs, axis=0)


# revision 12
# speedup vs baseline: 1.9475x; 1.3835x over previous
"""Trainium2 Bass kernel for windowed (sparse) cross-attention.

Computation (per batch row b of x:(B=2048, N=64, D=512)):
  q/k/v = x @ Wq/Wk/Wv, split into 8 heads of dim 64.
  13 temporal windows of size 16, stride 4 over N=64; softmax attention within
  each window; overlapping window outputs are accumulated and divided by the
  per-position window count; out = value @ Wout + bout.

Strategy (pure data parallel over 8 NeuronCores, batch-sharded):
  - Host pre-transposes the x shard to xT (D, T) and casts operands to bf16.
  - Projections produce qT/kT (inner-on-partitions) and v (tokens-on-partitions).
  - Scores S'[m,n] = k_m . q_n are computed per (2-batch x 2-head) "quad" with
    K=64 matmuls; the full 64x64 score block per pair is materialized (windows
    are diagonal 16x16 sub-blocks of it).
  - Window softmax is linear-algebra-ified: with E = exp(S*scale),
      window sums   s[w, n] = (MaskStack^T @ E)        (one matmul per unit,
                                                        packed 4 units/bank)
      1/s           via exp(-ln(s)) on the ACT engine  (ln+exp share one act
                                                        table set: no swaps)
      R'[m, n] = MaskStack @ (U * 1/s)                 (one matmul)
      P'[m, n] = E * R'                                (elementwise)
    where U[w,n] = 1[n in window w] / cnt[n].  Then value^T = v^T-contracted
    AV matmuls over P' columns.
  - Four units' window sums are packed into one PSUM bank at 32-aligned
    partition offsets (mask padded to 32 dummy columns so every partition
    holds a valid positive sum); ln/exp then run once per 4 units.  The
    U-multiply writes per-unit tiles at base partition 0 so matmul operands
    never sit at a partition offset (offset operands wedge this hardware).
  - Output projection consumes value^T chunks as stationary operands and
    produces the output in natural (token, D) layout; bias added on DVE.
"""

import sys

if "/opt/trn_rl_repo" not in sys.path:
    sys.path.insert(0, "/opt/trn_rl_repo")

import numpy as np
import ml_dtypes

import concourse.bass as bass
import concourse.tile as tile
from concourse import mybir
from concourse.bass_utils import run_bass_kernel_spmd

BF16 = mybir.dt.bfloat16
F32 = mybir.dt.float32
NP_BF16 = ml_dtypes.bfloat16

# Problem constants (hardcoded per contract)
B, N, D = 2048, 64, 512
NCORES = 8
BC = B // NCORES          # batch rows per core
T_FULL = BC * N           # tokens per core = 16384
HEADS, DH = 8, 64
WINDOW, STRIDE, NW = 16, 4, 13
SCALE = DH ** -0.5
TB = 512                  # tokens per block (8 batch rows)

# stash for test harness introspection
last_results = None


def _split_waits(nc, keep=1):
    """walrus in this toolchain supports only one embedded sync wait per
    instruction; hoist excess waits onto standalone EventSemaphore
    instructions on the same engine queue (FIFO => executes first)."""
    ctr = 0
    for f in nc.m.functions:
        for blk in f.blocks:
            il = blk.instructions
            out = []
            changed = False
            for inst in il:
                si = inst.sync_info
                if si is not None and len(si.on_wait) > keep:
                    waits = list(si.on_wait)
                    SyncInfo = type(si)
                    for w in waits[:-keep]:
                        evs = mybir.InstEventSemaphore(
                            name=f"WSPLIT-{ctr}", ins=[], outs=[]
                        )
                        ctr += 1
                        evs.engine = inst.engine
                        evs.sync_info = SyncInfo(on_wait=[w], on_update=[])
                        out.append(evs)
                    inst.sync_info = SyncInfo(
                        on_wait=waits[-keep:], on_update=list(si.on_update)
                    )
                    changed = True
                out.append(inst)
            if changed:
                il[:] = out
    return ctr


def _window_consts():
    idx = np.arange(NW)[:, None] * STRIDE + np.arange(WINDOW)[None, :]
    cnt = np.zeros(N, dtype=np.float64)
    np.add.at(cnt, idx, 1.0)
    member = np.zeros((N, NW), dtype=np.float64)  # member[m, w] = m in window w
    for w in range(NW):
        member[idx[w], w] = 1.0
    # 32-column mask: 26 real windows + 6 dummy copies of window 0 so that a
    # packed 4-unit [128, 512] window-sum bank has every partition written
    # with a valid positive sum (safe for ln).
    mask32 = np.zeros((128, 32), dtype=np.float64)
    mask32[:64, :13] = member
    mask32[64:, 13:26] = member
    for j in range(26, 32):
        mask32[:, j] = mask32[:, 0]
    mask_t = np.zeros((26, 128), dtype=np.float64)
    mask_t[:13, :64] = member.T
    mask_t[13:, 64:] = member.T
    u = np.zeros((26, 512), dtype=np.float64)
    for j in range(512):
        s = ((j % 256) // 64) % 2
        n = j % 64
        u[s * 13:(s + 1) * 13, j] = member[n] / cnt[n]
    # replicate u into each 32-aligned partition band: DVE tensor_tensor
    # requires both SBUF inputs at the same base partition
    u4 = np.zeros((128, 512), dtype=np.float64)
    for i in range(4):
        u4[i * 32:i * 32 + 26] = u
    return (
        mask32.astype(NP_BF16),
        mask_t.astype(NP_BF16),
        u4.astype(NP_BF16),
    )


def build_program(T=T_FULL):
    nc = bass.Bass()
    xt_d = nc.dram_tensor("xt", [D, T], BF16, kind="ExternalInput")
    wq_d = nc.dram_tensor("wq", [128, 4, D], BF16, kind="ExternalInput")
    wk_d = nc.dram_tensor("wk", [128, 4, D], BF16, kind="ExternalInput")
    wv_d = nc.dram_tensor("wv", [128, 4, D], BF16, kind="ExternalInput")
    wo_d = nc.dram_tensor("wo", [128, 4, D], BF16, kind="ExternalInput")
    bo_d = nc.dram_tensor("bo", [128, D], F32, kind="ExternalInput")
    ms_d = nc.dram_tensor("ms", [128, 32], BF16, kind="ExternalInput")
    mt_d = nc.dram_tensor("mt", [26, 128], BF16, kind="ExternalInput")
    u_d = nc.dram_tensor("u", [128, 512], BF16, kind="ExternalInput")
    out_d = nc.dram_tensor("out", [T, D], F32, kind="ExternalOutput")

    NB = T // TB
    EXP = mybir.ActivationFunctionType.Exp
    LN = mybir.ActivationFunctionType.Ln

    with tile.TileContext(nc) as tc:
        with (
            tc.tile_pool(name="consts", bufs=1) as consts,
            tc.tile_pool(name="xtp", bufs=8) as xt_pool,
            tc.tile_pool(name="qkp", bufs=16) as qk_pool,
            tc.tile_pool(name="vp", bufs=8) as v_pool,
            tc.tile_pool(name="ep", bufs=6) as e_pool,
            tc.tile_pool(name="lnp", bufs=2) as ln_pool,
            tc.tile_pool(name="rcp", bufs=8) as rc_pool,
            tc.tile_pool(name="pp", bufs=4) as p_pool,
            tc.tile_pool(name="vtp", bufs=8) as vt_pool,
            tc.tile_pool(name="op", bufs=4) as out_pool,
            tc.tile_pool(name="ps_proj", bufs=2, space="PSUM") as ps_proj,
            tc.tile_pool(name="ps_out", bufs=1, space="PSUM") as ps_out,
            tc.tile_pool(name="ps_s", bufs=2, space="PSUM") as ps_s,
            tc.tile_pool(name="ps_wr", bufs=2, space="PSUM") as ps_wr,
            tc.tile_pool(name="ps_av", bufs=1, space="PSUM") as ps_av,
        ):
            wq_t = consts.tile([128, 4, D], BF16, tag="wq")
            nc.sync.dma_start(wq_t[:], wq_d[:])
            wk_t = consts.tile([128, 4, D], BF16, tag="wk")
            nc.sync.dma_start(wk_t[:], wk_d[:])
            wv_t = consts.tile([128, 4, D], BF16, tag="wv")
            nc.sync.dma_start(wv_t[:], wv_d[:])
            wo_t = consts.tile([128, 4, D], BF16, tag="wo")
            nc.sync.dma_start(wo_t[:], wo_d[:])
            bo_t = consts.tile([128, D], F32, tag="bo")
            nc.sync.dma_start(bo_t[:], bo_d[:])
            ms_t = consts.tile([128, 32], BF16, tag="ms")
            nc.sync.dma_start(ms_t[:], ms_d[:])
            mt_t = consts.tile([26, 128], BF16, tag="mt")
            nc.sync.dma_start(mt_t[:], mt_d[:])
            u_t = consts.tile([128, 512], BF16, tag="u")
            nc.sync.dma_start(u_t[:], u_d[:])

            for blk in range(NB):
                t0 = blk * TB

                # ---- load xT tiles (D on partitions, 4 chunks) ----
                xts = []
                for kc in range(4):
                    xt_t = xt_pool.tile([128, TB], BF16, tag="xt")
                    nc.sync.dma_start(
                        xt_t[:], xt_d[kc * 128:(kc + 1) * 128, t0:t0 + TB]
                    )
                    xts.append(xt_t)

                # ---- qT / kT projections, stored as per-head-half tiles
                # [64, TB] at base partition 0 (avoids partition-offset
                # matmul operands, which wedge this hardware) ----
                qts, kts = [], []
                for wt, lst in ((wq_t, qts), (wk_t, kts)):
                    for c in range(4):
                        ps = ps_proj.tile([128, TB], F32, tag="pp")
                        for kc in range(4):
                            nc.tensor.matmul(
                                ps[:],
                                wt[:, kc, c * 128:(c + 1) * 128],
                                xts[kc][:],
                                start=(kc == 0),
                                stop=(kc == 3),
                            )
                        halves = []
                        for hh in range(2):
                            sb = qk_pool.tile([64, TB], BF16, tag="qk")
                            nc.scalar.copy(sb[:], ps[hh * 64:(hh + 1) * 64, :])
                            halves.append(sb)
                        lst.append(halves)

                # ---- v projection: natural layout [128 tokens, 512 i] ----
                vts = []
                for tt in range(4):
                    ps = ps_proj.tile([128, 512], F32, tag="pp")
                    for kc in range(4):
                        nc.tensor.matmul(
                            ps[:],
                            xts[kc][:, tt * 128:(tt + 1) * 128],
                            wv_t[:, kc, :],
                            start=(kc == 0),
                            stop=(kc == 3),
                        )
                    sb = v_pool.tile([128, 512], BF16, tag="vv")
                    nc.vector.tensor_copy(sb[:], ps[:])
                    vts.append(sb)

                # ---- attention, two groups of 4 units (unit = 2 token-pairs
                # x 2 heads); each group spans 2 head-chunks ----
                vt_out = [None] * 4
                for g in range(2):
                    units = [(2 * g + (ui >> 1), ui & 1) for ui in range(4)]
                    swb = ps_wr.tile([128, 512], F32, tag="swrp")
                    eus = []
                    for ui, (c, tb2) in enumerate(units):
                        qc, kc_t = qts[c], kts[c]
                        sp = ps_s.tile([128, 512], F32, tag="sp")
                        for qd in range(2):
                            tb = tb2 * 2 + qd
                            for hh in range(2):
                                tcols = slice(tb * 128, (tb + 1) * 128)
                                o = sp[:, qd * 256 + hh * 128:
                                       qd * 256 + (hh + 1) * 128]
                                nc.tensor.matmul(
                                    o, kc_t[hh][:, tcols], qc[hh][:, tcols],
                                    start=True, stop=True,
                                )
                        eu = e_pool.tile([128, 512], BF16, tag="eu")
                        nc.scalar.activation(eu[:], sp[:], EXP, scale=float(SCALE))
                        eus.append(eu)
                        # window sums for this unit -> 32-aligned partition
                        # band of the shared bank
                        nc.tensor.matmul(
                            swb[ui * 32:(ui + 1) * 32, :], ms_t[:], eu[:],
                            start=True, stop=True,
                            tile_position=(0, ui * 32),
                        )
                    # 1/s for all 4 units: exp(-ln(s)); both funcs live in the
                    # same ACT table set, so no table reloads
                    lt = ln_pool.tile([128, 512], F32, tag="lt")
                    nc.scalar.activation(lt[:], swb[:], LN)
                    et = ln_pool.tile([128, 512], BF16, tag="et")
                    nc.scalar.activation(et[:], lt[:], EXP, scale=-1.0)

                    av_cur = None
                    for ui, (c, tb2) in enumerate(units):
                        rcu = rc_pool.tile([26, 512], BF16, tag="rcu")
                        nc.vector.tensor_mul(
                            rcu[:], et[ui * 32:ui * 32 + 26, :],
                            u_t[ui * 32:ui * 32 + 26, :],
                        )
                        rp = ps_wr.tile([128, 512], F32, tag="swrp")
                        nc.tensor.matmul(rp[:], mt_t[:], rcu[:], start=True, stop=True)
                        pu = p_pool.tile([128, 512], BF16, tag="pu")
                        nc.vector.tensor_mul(pu[:], eus[ui][:], rp[:])
                        if tb2 == 0:
                            av_cur = ps_av.tile([128, 512], F32, tag="av")
                        av = av_cur
                        for qd in range(2):
                            tb = tb2 * 2 + qd
                            for hh in range(2):
                                lhsT = vts[tb][
                                    :, c * 128 + hh * 64: c * 128 + hh * 64 + 64
                                ]
                                rhs = pu[:, qd * 256 + hh * 128:
                                         qd * 256 + (hh + 1) * 128]
                                o = av[hh * 64:(hh + 1) * 64,
                                       tb * 128:(tb + 1) * 128]
                                nc.tensor.matmul(o, lhsT, rhs, start=True, stop=True)
                        if tb2 == 1:
                            vt = vt_pool.tile([128, 512], BF16, tag="vt")
                            nc.scalar.copy(vt[:], av[:])
                            vt_out[c] = vt

                # ---- output projection + bias ----
                for tt in range(4):
                    ps = ps_out.tile([128, 512], F32, tag="po")
                    for c in range(4):
                        nc.tensor.matmul(
                            ps[:],
                            vt_out[c][:, tt * 128:(tt + 1) * 128],
                            wo_t[:, c, :],
                            start=(c == 0),
                            stop=(c == 3),
                        )
                    ob = out_pool.tile([128, 512], F32, tag="ob")
                    nc.vector.tensor_add(ob[:], ps[:], bo_t[:])
                    nc.sync.dma_start(
                        out_d[t0 + tt * 128: t0 + (tt + 1) * 128, :], ob[:]
                    )
    return nc


def _prep_shared(Wq, Wk, Wv, Wout, bout):
    def warr(w):
        return np.ascontiguousarray(
            w.astype(np.float32).reshape(4, 128, D).transpose(1, 0, 2)
        ).astype(NP_BF16)

    mask32, mask_t, u = _window_consts()
    return {
        "wq": warr(Wq),
        "wk": warr(Wk),
        "wv": warr(Wv),
        "wo": warr(Wout),
        "bo": np.ascontiguousarray(
            np.broadcast_to(bout.astype(np.float32), (128, D))
        ),
        "ms": mask32,
        "mt": mask_t,
        "u": u,
    }


def kernel(x, Wq, Wk, Wv, Wout, bout):
    global last_results
    x = np.asarray(x, dtype=np.float32)
    shared = _prep_shared(
        np.asarray(Wq), np.asarray(Wk), np.asarray(Wv),
        np.asarray(Wout), np.asarray(bout),
    )
    in_maps = []
    for ci in range(NCORES):
        xs = x[ci * BC:(ci + 1) * BC].reshape(T_FULL, D)
        xt = np.ascontiguousarray(xs.T).astype(NP_BF16)
        in_maps.append({"xt": xt, **shared})

    nc = build_program(T_FULL)
    _split_waits(nc)
    res = run_bass_kernel_spmd(nc, in_maps, list(range(NCORES)))
    last_results = res
    outs = [
        res.results[ci]["out"].astype(np.float32).reshape(BC, N, D)
        for ci in range(NCORES)
    ]
    return np.concatenate(outs, axis=0)
